# revision 1
# baseline (speedup 1.0000x reference)
"""Bass/Trainium2 kernel for nn_BiChannelAttention (single-query local-window attention).

Math (per batch b, head h, with S=2049, window W=256, cutoff=S-W=1793):
  Positions before the cutoff receive a -1e6 additive mask, so after softmax their
  weight is exactly 0.0 in fp32 (exp underflows). Only the last W positions matter.

  For window rows X [W, 128] (last 255 cache rows + content row):
    q   = cnt_h @ Wq_h                      (128)
    kq  = (Wk_h/sqrt(128))^T q              (128)      <- folds Wk into q
    sc  = X kq  (+ per-position bias)       (W)        <- column-major on chip
    a   = exp(sc)          (no max-subtraction needed: unmasked scores are O(1))
    xa  = X^T a / sum(a)                    (128)
    out = Wv_h^T xa + cnt_h                 (128)

Sharding: tensor-parallel over heads, 2 heads per core x 8 cores. Each core reads
only its heads' weight slices and window slices (~2.2 MB).
"""

import sys
import numpy as np

for _p in ("/opt/trn_rl_repo", "/root/.axon_site/_ro/trn_rl_repo"):
    if _p not in sys.path:
        sys.path.insert(0, _p)

import concourse.bass as bass
import concourse.bacc as bacc
import concourse.mybir as mybir
from concourse.tile import TileContext
from concourse.bass_utils import run_bass_kernel_spmd

F32 = mybir.dt.float32
P = 128          # partitions / head_dim
B = 8            # batch
H = 16           # heads total
HPC = 2          # heads per core
NCORES = 8
T = 2048
S = T + 1
W = 256          # local attention window
CUTOFF = S - W   # 1793
NEG = -1000000.0

_NC_CACHE = {}


def _build_nc():
    nc = bacc.Bacc(None, target_bir_lowering=False, debug=False)
    # packed constants along the free dim: ident | ones | bias | cnt | (wq,wkt,wv) x HPC
    CK = 2 * P + 2 * B + HPC * B + 3 * HPC * P
    x_in = nc.declare_dram_parameter("x", [B, HPC, W, P], F32, isOutput=False)
    consts_in = nc.declare_dram_parameter("consts", [P, CK], F32, isOutput=False)
    out_t = nc.declare_dram_parameter("out", [HPC, P, B], F32, isOutput=True)

    with TileContext(nc) as tc:
        with (
            tc.tile_pool(name="const", bufs=1) as cpool,
            tc.tile_pool(name="xin", bufs=10) as xpool,
            tc.tile_pool(name="xt", bufs=10) as xtpool,
            tc.tile_pool(name="small", bufs=2) as spool,
            tc.tile_pool(name="ps_t", bufs=2, space="PSUM") as pst,
            tc.tile_pool(name="ps_qk", bufs=2, space="PSUM") as psqk,
            tc.tile_pool(name="ps_at", bufs=2, space="PSUM") as psat,
            tc.tile_pool(name="ps_xo", bufs=2, space="PSUM") as psxo,
        ):
            consts = cpool.tile([P, CK], F32, tag="consts")
            nc.sync.dma_start(out=consts[:, :], in_=consts_in[:, :])
            o = 0
            ident = consts[:, o:o + P]; o += P
            ones = consts[:, o:o + P]; o += P
            biasT = consts[:, o:o + 2 * B]; o += 2 * B
            cntT = consts[:, o:o + HPC * B]; o += HPC * B
            wq, wkt, wv = [], [], []
            for j in range(HPC):
                wq.append(consts[:, o:o + P]); o += P
                wkt.append(consts[:, o:o + P]); o += P
                wv.append(consts[:, o:o + P]); o += P

            for j in range(HPC):
                cnt_j = cntT[:, j * B:(j + 1) * B]

                # q for all 8 batches: q[e,b] = sum_d Wq[d,e] cnt[d,b]
                qk_ps = psqk.tile([P, 2 * B], F32, tag="qk")
                nc.tensor.matmul(qk_ps[:, 0:B], wq[j], cnt_j, start=True, stop=True)
                q_sb = spool.tile([P, B], F32, tag="q")
                nc.vector.tensor_copy(q_sb[:, :], qk_ps[:, 0:B])
                # kq[d,b] = sum_e WkT[e,d] q[e,b]   (WkT pre-scaled by 1/sqrt(128))
                nc.tensor.matmul(qk_ps[:, B:2 * B], wkt[j], q_sb[:, :], start=True, stop=True)
                kq_sb = spool.tile([P, B], F32, tag="kq")
                nc.vector.tensor_copy(kq_sb[:, :], qk_ps[:, B:2 * B])

                at_ps = psat.tile([P, 3 * B], F32, tag="at")  # scores [0:16], denom [16:24]
                xo_ps = psxo.tile([P, 2 * B], F32, tag="xo")  # xa [0:8], out [8:16]

                x0s, x1s, xt0s, xt1s = [], [], [], []
                for b in range(B):
                    x0 = xpool.tile([P, P], F32, tag="x0")
                    nc.sync.dma_start(out=x0[:, :], in_=x_in[b, j, 0:P, :])
                    x1 = xpool.tile([P, P], F32, tag="x1")
                    nc.sync.dma_start(out=x1[:, :], in_=x_in[b, j, P:W, :])
                    xt_ps = pst.tile([P, 2 * P], F32, tag="xtp")
                    nc.tensor.transpose(xt_ps[:, 0:P], x0[:, :], ident)
                    nc.tensor.transpose(xt_ps[:, P:2 * P], x1[:, :], ident)
                    xt0 = xtpool.tile([P, P], F32, tag="xt0")
                    nc.vector.tensor_copy(xt0[:, :], xt_ps[:, 0:P])
                    xt1 = xtpool.tile([P, P], F32, tag="xt1")
                    nc.scalar.copy(xt1[:, :], xt_ps[:, P:2 * P])
                    # scores: column [s,1] per (tile, b) -> at_ps col jt*8+b
                    nc.tensor.matmul(at_ps[:, b:b + 1], xt0[:, :], kq_sb[:, b:b + 1], start=True, stop=True)
                    nc.tensor.matmul(at_ps[:, B + b:B + b + 1], xt1[:, :], kq_sb[:, b:b + 1], start=True, stop=True)
                    x0s.append(x0); x1s.append(x1); xt0s.append(xt0); xt1s.append(xt1)

                # bias add + exp for all 16 score columns at once
                att_pre = spool.tile([P, 2 * B], F32, tag="att_pre")
                nc.vector.tensor_add(att_pre[:, :], at_ps[:, 0:2 * B], biasT)
                att = spool.tile([P, 2 * B], F32, tag="att")
                nc.scalar.activation(att[:, :], att_pre[:, :], mybir.ActivationFunctionType.Exp)

                # denominator broadcast over partitions: accumulate both s-tiles on PE
                nc.tensor.matmul(at_ps[:, 2 * B:3 * B], ones, att[:, 0:B], start=True, stop=False)
                nc.tensor.matmul(at_ps[:, 2 * B:3 * B], ones, att[:, B:2 * B], start=False, stop=True)
                rec = spool.tile([P, B], F32, tag="rec")
                nc.vector.reciprocal(rec[:, :], at_ps[:, 2 * B:3 * B])

                # xa[d,b] = sum_s X[s,d] a[s,b]  (accumulate the two s-tiles)
                for b in range(B):
                    nc.tensor.matmul(xo_ps[:, b:b + 1], x0s[b][:, :], att[:, b:b + 1], start=True, stop=False)
                    nc.tensor.matmul(xo_ps[:, b:b + 1], x1s[b][:, :], att[:, B + b:B + b + 1], start=False, stop=True)
                xa_sb = spool.tile([P, B], F32, tag="xa")
                nc.vector.tensor_mul(xa_sb[:, :], xo_ps[:, 0:B], rec[:, :])

                # out[e,b] = sum_d Wv[d,e] xa[d,b]; residual add; store
                nc.tensor.matmul(xo_ps[:, B:2 * B], wv[j], xa_sb[:, :], start=True, stop=True)
                fin = spool.tile([P, B], F32, tag="fin")
                nc.vector.tensor_add(fin[:, :], xo_ps[:, B:2 * B], cnt_j)
                nc.sync.dma_start(out=out_t[j, :, :], in_=fin[:, :])
    nc.finalize()
    return nc


def _get_nc():
    if "nc" not in _NC_CACHE:
        _NC_CACHE["nc"] = _build_nc()
    return _NC_CACHE["nc"]


def _pos_bias_f32():
    """t5_position_bucket exactly as the reference computes it (same jnp ops on the
    in-process default jax backend), sliced to the window."""
    if "pos" not in _NC_CACHE:
        import jax.numpy as jnp
        NUM_BUCKETS, MAX_DISTANCE = 32, 128
        n = (S - 1) - jnp.arange(S)
        max_exact = NUM_BUCKETS // 2
        is_small = n < max_exact
        large = max_exact + (
            jnp.log(jnp.maximum(n, 1).astype(jnp.float32) / max_exact)
            / np.log(MAX_DISTANCE / max_exact)
            * (NUM_BUCKETS - max_exact)
        ).astype(jnp.int32)
        large = jnp.minimum(large, NUM_BUCKETS - 1)
        pos = jnp.where(is_small, n, large).astype(jnp.float32)
        _NC_CACHE["pos"] = np.asarray(pos)[CUTOFF:]  # [W]
    return _NC_CACHE["pos"]


def kernel(**inputs) -> np.ndarray:
    t = int(np.asarray(inputs["t"]))
    assert t == T, f"kernel hardcoded for t={T}, got {t}"
    content_t = np.ascontiguousarray(np.asarray(inputs["content_t"], dtype=np.float32))
    time_mask = np.asarray(inputs["time_mask"])
    cache = np.asarray(inputs["cache"], dtype=np.float32)
    Wq = np.asarray(inputs["Wq"], dtype=np.float32)
    Wk = np.asarray(inputs["Wk"], dtype=np.float32)
    Wv = np.asarray(inputs["Wv"], dtype=np.float32)
    pos_param = np.float32(np.asarray(inputs["pos_param"]))

    # Per-position additive bias for the window: -pos_param*bucket only.
    # The reference's masked_fill sequence (1->0, then every 0->NEG) sets ALL
    # positions to NEG, a uniform shift softmax cancels -- time_mask is a no-op.
    del time_mask
    pos = _pos_bias_f32()                                   # [W]
    posb = (-pos_param * pos).astype(np.float32)            # [W]
    bias_col = posb.reshape(2, P).transpose(1, 0)           # [p, jt]
    bias_t = np.ascontiguousarray(
        np.broadcast_to(bias_col[:, :, None], (P, 2, B)).reshape(P, 2 * B)
    )  # [p, jt*8+b]

    win = cache[:, CUTOFF:T, :].reshape(B, W - 1, H, P)      # [B, 255, H, 128]
    cnt_h = content_t.reshape(B, H, P)                       # [B, H, 128]
    wkt_full = (Wk.transpose(0, 2, 1) / np.float32(np.sqrt(128.0))).astype(np.float32)

    ones = np.ones((P, P), np.float32)
    ident = np.eye(P, dtype=np.float32)

    in_maps = []
    for c in range(NCORES):
        h0 = HPC * c
        x_host = np.empty((B, HPC, W, P), np.float32)
        for j in range(HPC):
            x_host[:, j, : W - 1, :] = win[:, :, h0 + j, :]
            x_host[:, j, W - 1, :] = cnt_h[:, h0 + j, :]
        cnt_host = np.ascontiguousarray(
            cnt_h[:, h0:h0 + HPC, :].transpose(2, 1, 0).reshape(P, HPC * B)
        )  # [d, j*8+b]
        blocks = [ident, ones, bias_t, cnt_host]
        for j in range(HPC):
            blocks += [Wq[h0 + j], wkt_full[h0 + j], Wv[h0 + j]]
        consts_host = np.ascontiguousarray(np.concatenate(blocks, axis=1), dtype=np.float32)
        in_maps.append({"x": x_host, "consts": consts_host})

    nc = _get_nc()
    res = run_bass_kernel_spmd(nc, in_maps, list(range(NCORES)), **_RUN_KWARGS)
    _NC_CACHE["last_results"] = res
    outs = np.stack([np.asarray(res.results[c]["out"]) for c in range(NCORES)])
    # outs: [core, j, d, b] -> out_full[b, (2c+j)*128 + d]
    out_full = outs.transpose(3, 0, 1, 2).reshape(B, H * P)
    return out_full.astype(np.float32)


_RUN_KWARGS = {}  # test harness may set {"trace": True, "tmpdir": ...}



# revision 4
# speedup vs baseline: 2.6987x; 2.6987x over previous
"""Bass/Trainium2 kernel for nn_BiChannelAttention (single-query local-window attention).

Math (per batch b, head h, S=2049, window W=256, cutoff=S-W=1793):
  Positions before the cutoff get a -1e6 additive mask -> softmax weight exactly 0
  in fp32. Only the last W positions matter. The time_mask is a no-op (the
  reference's masked_fill chain shifts every score by the same -1e6).

  Window rows X [W=256, 128] (last 255 cache rows + content row):
    q    = Wq_h^T cnt_h                                  [128]
    kq   = (256/sqrt(128)) * Wk_h kq-fold:  kq = Wk_h q  [128]   (scaled x256 for fp8)
    sc   = X kq  (+ 256*bias)                            [256]   (stored as 256*score)
    a    = exp(sc/256)            (scores are O(0.05); no max-subtraction needed)
    xa   = X^T a ;  den = sum(a)
    out  = Wv_h^T xa / den + cnt_h

Precision: window X and attention weights ride in fp8e4m3 (scores are tiny so
softmax is insensitive; the output is dominated by the residual, and fp8 errors
average across the 256-wide near-uniform attention). Weights/q/kq-chain in bf16,
accumulation in fp32 PSUM. Measured rel err ~1e-3 vs the 2e-2 gate.

PE work per head: 2 (q,kq) + 16 score matvecs (stationary xt tile [d,s], fp8)
+ 1 ld ones + 2 den matmuls + 8 xa DoubleRow matmuls (contraction 256 = 2
k-tiles of 128) + 1 output projection. No on-chip transposes: the host ships
the window in BOTH layouts ([d,s] for scores, [s,t,d] for xa) as fp8, so total
DMA is ~1.3 MB/core in a few large contiguous transfers.

Sharding: tensor-parallel over heads, 2 heads per core x 8 cores.
"""

import sys
import numpy as np
import ml_dtypes

for _p in ("/opt/trn_rl_repo", "/root/.axon_site/_ro/trn_rl_repo"):
    if _p not in sys.path:
        sys.path.insert(0, _p)

import concourse.bass as bass
import concourse.bacc as bacc
import concourse.mybir as mybir
from concourse.tile import TileContext
from concourse.bass_utils import run_bass_kernel_spmd

F32 = mybir.dt.float32
BF16 = mybir.dt.bfloat16
F8 = mybir.dt.float8e4
NP_F8 = ml_dtypes.float8_e4m3
NP_BF16 = ml_dtypes.bfloat16

P = 128          # partitions / head_dim
B = 8            # batch
H = 16           # heads total
HPC = 2          # heads per core
NCORES = 8
T = 2048
S = T + 1
W = 256          # local attention window
NT = 2           # s-tiles per window
CUTOFF = S - W   # 1793
KSCALE = 256.0   # fp8 dynamic-range scale folded into wkt (and undone in exp)

_NC_CACHE = {}


def _build_nc():
    nc = bacc.Bacc(None, target_bir_lowering=False, debug=False)
    # xt: [j, d, b*W+s] fp8 -- scores stationary tiles [d, s]
    xt_in = nc.declare_dram_parameter("xt", [HPC, P, B * W], F8, isOutput=False)
    # x: [j, s_lo, b, t, d] fp8 -- xa DoubleRow stationary [s_lo, 2, d]
    x_in = nc.declare_dram_parameter("x", [HPC, P, B, NT, P], F8, isOutput=False)
    # bf16 consts: wq0|wkt0|wv0|wq1|wkt1|wv1|cnt(j*8+b)
    cbf_in = nc.declare_dram_parameter("cbf", [P, 6 * P + HPC * B], BF16, isOutput=False)
    # f32 consts: bias256 [p, b, t] (=256 * positional bias, same for all b)
    bias_in = nc.declare_dram_parameter("bias", [P, B, NT], F32, isOutput=False)
    # f32 consts: residual content [p, j, b]
    cnt_in = nc.declare_dram_parameter("cntf", [P, HPC, B], F32, isOutput=False)
    out_t = nc.declare_dram_parameter("out", [P, HPC * B], F32, isOutput=True)

    with TileContext(nc) as tc:
        with (
            tc.tile_pool(name="const", bufs=1) as cpool,
            tc.tile_pool(name="data", bufs=2) as dpool,
            tc.tile_pool(name="sm", bufs=2) as spool,
            tc.tile_pool(name="ps_a", bufs=2, space="PSUM") as psa,
            tc.tile_pool(name="ps_sc", bufs=2, space="PSUM") as pssc,
            tc.tile_pool(name="ps_b", bufs=2, space="PSUM") as psb,
        ):
            cbf = cpool.tile([P, 6 * P + HPC * B], BF16, tag="cbf")
            nc.sync.dma_start(out=cbf[:, :], in_=cbf_in[:, :])
            bias_sb = cpool.tile([P, B, NT], F32, tag="bias")
            nc.sync.dma_start(out=bias_sb[:, :, :], in_=bias_in[:, :, :])
            cnt_sb = cpool.tile([P, HPC, B], F32, tag="cntf")
            nc.sync.dma_start(out=cnt_sb[:, :, :], in_=cnt_in[:, :, :])
            ones8 = cpool.tile([P, P], F8, tag="ones")
            nc.gpsimd.memset(ones8[:, :], 1.0)

            wq = [cbf[:, (3 * j) * P:(3 * j + 1) * P] for j in range(HPC)]
            wkt = [cbf[:, (3 * j + 1) * P:(3 * j + 2) * P] for j in range(HPC)]
            wv = [cbf[:, (3 * j + 2) * P:(3 * j + 3) * P] for j in range(HPC)]
            cnt_bf = cbf[:, 6 * P:]

            xt_sb, x_sb = [], []
            for j in range(HPC):
                xt_j = dpool.tile([P, B * W], F8, tag="xt")
                nc.sync.dma_start(out=xt_j[:, :], in_=xt_in[j, :, :])
                x_j = dpool.tile([P, B, NT, P], F8, tag="x")
                nc.sync.dma_start(out=x_j[:, :, :, :], in_=x_in[j, :, :, :, :])
                xt_sb.append(xt_j)
                x_sb.append(x_j)

            # Phase A per head: q, kq, scores, exp  (A0, A1 interleave so PE
            # streams head1 scores while head0's exp runs on scalar engine)
            att = []
            for j in range(HPC):
                qk_ps = psa.tile([P, 2 * B], F32, tag="qk")
                nc.tensor.matmul(qk_ps[:, 0:B], wq[j], cnt_bf[:, j * B:(j + 1) * B],
                                 start=True, stop=True)
                q_bf = spool.tile([P, B], BF16, tag="q")
                nc.vector.tensor_copy(q_bf[:, :], qk_ps[:, 0:B])
                nc.tensor.matmul(qk_ps[:, B:2 * B], wkt[j], q_bf[:, :],
                                 start=True, stop=True)
                kq8 = spool.tile([P, B], F8, tag="kq")
                nc.vector.tensor_copy(kq8[:, :], qk_ps[:, B:2 * B])

                sc_ps = pssc.tile([P, B, NT], F32, tag="sc")
                for b in range(B):
                    for t in range(NT):
                        nc.tensor.matmul(
                            sc_ps[:, b, t:t + 1],
                            xt_sb[j][:, b * W + t * P: b * W + (t + 1) * P],
                            kq8[:, b:b + 1],
                            start=True, stop=True,
                        )
                pre = spool.tile([P, B, NT], F32, tag="pre")
                nc.vector.tensor_add(pre[:, :, :], sc_ps[:, :, :], bias_sb[:, :, :])
                att_j = spool.tile([P, B, NT], F8, tag="att")
                nc.scalar.activation(att_j[:, :, :], pre[:, :, :],
                                     mybir.ActivationFunctionType.Exp,
                                     scale=1.0 / KSCALE)
                att.append(att_j)

            # Phase B per head: denominator, xa (DoubleRow), output projection
            for j in range(HPC):
                b_ps = psb.tile([P, 3 * B], F32, tag="bps")
                den_ps = b_ps[:, 0:B]
                xa_ps = b_ps[:, B:2 * B]
                o_ps = b_ps[:, 2 * B:3 * B]
                nc.tensor.matmul(den_ps, ones8, att[j][:, :, 0],
                                 start=True, stop=False)
                nc.tensor.matmul(den_ps, ones8, att[j][:, :, 1],
                                 start=False, stop=True)
                rec = spool.tile([P, B], F32, tag="rec")
                nc.vector.reciprocal(rec[:, :], den_ps)

                for b in range(B):
                    nc.tensor.matmul(
                        xa_ps[:, b:b + 1],
                        x_sb[j][:, b],                       # [128, 2, 128]
                        att[j][:, b].unsqueeze(2),           # [128, 2, 1]
                        start=True, stop=True,
                        perf_mode=mybir.MatmulPerfMode.DoubleRow,
                    )
                xa_bf = spool.tile([P, B], BF16, tag="xab")
                nc.vector.tensor_copy(xa_bf[:, :], xa_ps)

                nc.tensor.matmul(o_ps, wv[j], xa_bf[:, :], start=True, stop=True)
                t1 = spool.tile([P, B], F32, tag="t1")
                nc.vector.tensor_mul(t1[:, :], o_ps, rec[:, :])
                fin = spool.tile([P, B], F32, tag="fin")
                nc.gpsimd.tensor_add(fin[:, :], t1[:, :], cnt_sb[:, j, :])
                nc.sync.dma_start(out=out_t[:, j * B:(j + 1) * B], in_=fin[:, :])
    nc.finalize()
    return nc


def _get_nc():
    if "nc" not in _NC_CACHE:
        _NC_CACHE["nc"] = _build_nc()
    return _NC_CACHE["nc"]


def _pos_bias_f32():
    """t5_position_bucket exactly as the reference computes it, sliced to the
    window."""
    if "pos" not in _NC_CACHE:
        import jax.numpy as jnp
        NUM_BUCKETS, MAX_DISTANCE = 32, 128
        n = (S - 1) - jnp.arange(S)
        max_exact = NUM_BUCKETS // 2
        is_small = n < max_exact
        large = max_exact + (
            jnp.log(jnp.maximum(n, 1).astype(jnp.float32) / max_exact)
            / np.log(MAX_DISTANCE / max_exact)
            * (NUM_BUCKETS - max_exact)
        ).astype(jnp.int32)
        large = jnp.minimum(large, NUM_BUCKETS - 1)
        pos = jnp.where(is_small, n, large).astype(jnp.float32)
        _NC_CACHE["pos"] = np.asarray(pos)[CUTOFF:]  # [W]
    return _NC_CACHE["pos"]


def kernel(**inputs) -> np.ndarray:
    t = int(np.asarray(inputs["t"]))
    assert t == T, f"kernel hardcoded for t={T}, got {t}"
    content_t = np.ascontiguousarray(np.asarray(inputs["content_t"], dtype=np.float32))
    cache = np.asarray(inputs["cache"], dtype=np.float32)
    Wq = np.asarray(inputs["Wq"], dtype=np.float32)
    Wk = np.asarray(inputs["Wk"], dtype=np.float32)
    Wv = np.asarray(inputs["Wv"], dtype=np.float32)
    pos_param = np.float32(np.asarray(inputs["pos_param"]))

    pos = _pos_bias_f32()                                   # [W]
    posb = (-KSCALE * pos_param * pos).astype(np.float32)   # [W], pre-scaled x256
    # bias tile [p, b, t]: value depends on (t, p) only
    bias_host = np.ascontiguousarray(
        np.broadcast_to(posb.reshape(NT, P).transpose(1, 0)[:, None, :], (P, B, NT))
    ).astype(np.float32)

    cnt_h = content_t.reshape(B, H, P)                      # [B, H, 128]
    # full window per (b, h): last 255 cache rows + content row
    win = np.empty((B, H, W, P), np.float32)
    win[:, :, : W - 1, :] = cache[:, CUTOFF:T, :].reshape(B, W - 1, H, P).transpose(0, 2, 1, 3)
    win[:, :, W - 1, :] = cnt_h
    win8 = win.astype(NP_F8)                                # [B, H, 256, 128] fp8

    wkt_full = (Wk.transpose(0, 2, 1) * np.float32(KSCALE / np.sqrt(128.0))).astype(np.float32)

    in_maps = []
    for c in range(NCORES):
        h0 = HPC * c
        xt_host = np.empty((HPC, P, B * W), NP_F8)
        x_host = np.empty((HPC, P, B, NT, P), NP_F8)
        for j in range(HPC):
            wj = win8[:, h0 + j]                            # [B, 256, 128]
            xt_host[j] = wj.transpose(2, 0, 1).reshape(P, B * W)
            x_host[j] = wj.reshape(B, NT, P, P).transpose(2, 0, 1, 3)
        blocks = []
        for j in range(HPC):
            blocks += [Wq[h0 + j], wkt_full[h0 + j], Wv[h0 + j]]
        blocks.append(cnt_h[:, h0:h0 + HPC, :].transpose(2, 1, 0).reshape(P, HPC * B))
        cbf_host = np.concatenate(blocks, axis=1).astype(NP_BF16)
        cnt_host = np.ascontiguousarray(
            cnt_h[:, h0:h0 + HPC, :].transpose(2, 1, 0)
        ).astype(np.float32)                                # [p, j, b]
        in_maps.append({
            "xt": xt_host, "x": x_host, "cbf": cbf_host,
            "bias": bias_host, "cntf": cnt_host,
        })

    nc = _get_nc()
    res = run_bass_kernel_spmd(nc, in_maps, list(range(NCORES)), **_RUN_KWARGS)
    _NC_CACHE["last_results"] = res
    outs = np.stack([np.asarray(res.results[c]["out"]) for c in range(NCORES)])
    # outs: [core, d, j*8+b] -> out_full[b, (2c+j)*128 + d]
    out_full = outs.reshape(NCORES, P, HPC, B).transpose(3, 0, 2, 1).reshape(B, H * P)
    return out_full.astype(np.float32)


_RUN_KWARGS = {}  # test harness may set {"trace": True, "tmpdir": ...}


# revision 9
# speedup vs baseline: 2.8397x; 1.0523x over previous
"""Bass/Trainium2 kernel for nn_BiChannelAttention (single-query local-window attention).

Math (per batch b, head h, S=2049, window W=256, cutoff=S-W=1793):
  Positions before the cutoff get a -1e6 additive mask -> softmax weight exactly 0
  in fp32. Only the last W positions matter. The time_mask is a no-op (the
  reference's masked_fill chain shifts every score by the same -1e6).

  Window rows X [W=256, 128] (last 255 cache rows + content row):
    q    = Wq_h^T cnt_h                                  [128]
    kq   = (256/sqrt(128)) * Wk_h kq-fold:  kq = Wk_h q  [128]   (scaled x256 for fp8)
    sc   = X kq  (+ 256*bias)                            [256]   (stored as 256*score)
    a    = exp(sc/256)            (scores are O(0.05); no max-subtraction needed)
    xa   = X^T a ;  den = sum(a)
    out  = Wv_h^T xa / den + cnt_h

Precision: window X and attention weights ride in fp8e4m3 (scores are tiny so
softmax is insensitive; the output is dominated by the residual, and fp8 errors
average across the 256-wide near-uniform attention). Weights/q/kq-chain in bf16,
accumulation in fp32 PSUM. Measured rel err ~1e-3 vs the 2e-2 gate.

PE work per head: 2 (q,kq) + 16 score matvecs (stationary xt tile [d,s], fp8)
+ 1 ld ones + 2 den matmuls + 8 xa DoubleRow matmuls (contraction 256 = 2
k-tiles of 128) + 1 output projection. No on-chip transposes: the host ships
the window in BOTH layouts ([d,s] for scores, [s,t,d] for xa) as fp8, so total
DMA is ~1.3 MB/core in a few large contiguous transfers.

Sharding: tensor-parallel over heads, 2 heads per core x 8 cores.
"""

import sys
import numpy as np
import ml_dtypes

for _p in ("/opt/trn_rl_repo", "/root/.axon_site/_ro/trn_rl_repo"):
    if _p not in sys.path:
        sys.path.insert(0, _p)

import concourse.bass as bass
import concourse.bacc as bacc
import concourse.mybir as mybir
from concourse.tile import TileContext
from concourse.bass_utils import run_bass_kernel_spmd

F32 = mybir.dt.float32
BF16 = mybir.dt.bfloat16
F8 = mybir.dt.float8e4
NP_F8 = ml_dtypes.float8_e4m3
NP_BF16 = ml_dtypes.bfloat16

P = 128          # partitions / head_dim
B = 8            # batch
H = 16           # heads total
HPC = 2          # heads per core
NCORES = 8
T = 2048
S = T + 1
W = 256          # local attention window
NT = 2           # s-tiles per window
CUTOFF = S - W   # 1793
KSCALE = 256.0   # fp8 dynamic-range scale folded into wkt (and undone in exp)

_NC_CACHE = {}


def _build_nc():
    nc = bacc.Bacc(None, target_bir_lowering=False, debug=False)
    # xt: [j, d, b*W+s] fp8 -- scores stationary tiles [d, s]
    xt_in = nc.declare_dram_parameter("xt", [HPC, P, B * W], F8, isOutput=False)
    # x: [j, s_lo, b, t, d] fp8 -- xa stationary tiles [s_lo, d]
    x_in = nc.declare_dram_parameter("x", [HPC, P, B, NT, P], F8, isOutput=False)
    # bf16 consts: mfold0|wv0|mfold1|wv1|cnt(j*8+b); mfold = 256/sqrt(128)*Wq@Wk^T
    cbf_in = nc.declare_dram_parameter("cbf", [P, 4 * P + HPC * B], BF16, isOutput=False)
    # f32 consts: bias [p, t] (positional, unscaled) | residual content [p, j*8+b]
    cf_in = nc.declare_dram_parameter("cf", [P, NT + HPC * B], F32, isOutput=False)
    out_t = nc.declare_dram_parameter("out", [P, HPC * B], F32, isOutput=True)

    with TileContext(nc) as tc:
        with (
            tc.tile_pool(name="const", bufs=1) as cpool,
            tc.tile_pool(name="data", bufs=2) as dpool,
            tc.tile_pool(name="sm", bufs=2) as spool,
            tc.tile_pool(name="ps_a", bufs=2, space="PSUM") as psa,
            tc.tile_pool(name="ps_sc", bufs=2, space="PSUM") as pssc,
            tc.tile_pool(name="ps_b", bufs=2, space="PSUM") as psb,
        ):
            # big window tensors first, each dispatched from its own engine
            # queue so the four transfers overlap
            xt_sb, x_sb = [], []
            xt_sb.append(dpool.tile([P, B * W], F8, tag="xt", name="xt0"))
            x_sb.append(dpool.tile([P, B, NT, P], F8, tag="x", name="x0"))
            xt_sb.append(dpool.tile([P, B * W], F8, tag="xt", name="xt1"))
            x_sb.append(dpool.tile([P, B, NT, P], F8, tag="x", name="x1"))
            nc.gpsimd.dma_start(out=xt_sb[0][:, :], in_=xt_in[0, :, :])
            nc.scalar.dma_start(out=xt_sb[1][:, :], in_=xt_in[1, :, :])
            nc.scalar.dma_start(out=x_sb[0][:, :, :, :], in_=x_in[0, :, :, :, :])
            nc.gpsimd.dma_start(out=x_sb[1][:, :, :, :], in_=x_in[1, :, :, :, :])

            cbf = cpool.tile([P, 4 * P + HPC * B], BF16, tag="cbf")
            nc.sync.dma_start(out=cbf[:, :], in_=cbf_in[:, :])
            cf = cpool.tile([P, NT + HPC * B], F32, tag="cf")
            nc.sync.dma_start(out=cf[:, :], in_=cf_in[:, :])
            ones8 = cpool.tile([P, P], F8, tag="ones")
            nc.gpsimd.memset(ones8[:, :], 1.0)

            mfold = [cbf[:, (2 * j) * P:(2 * j + 1) * P] for j in range(HPC)]
            wv = [cbf[:, (2 * j + 1) * P:(2 * j + 2) * P] for j in range(HPC)]
            cnt_bf = cbf[:, 4 * P:]
            bias_sb = cf[:, 0:NT]
            cnt_f = cf[:, NT:]

            # Phase A per head: kq (folded), scores, exp  (A0, A1 interleave so
            # PE streams head1 scores while head0's exp runs on scalar engine)
            att = []
            for j in range(HPC):
                kq_ps = psa.tile([P, B], F32, tag="kq")
                nc.tensor.matmul(kq_ps[:, :], mfold[j], cnt_bf[:, j * B:(j + 1) * B],
                                 start=True, stop=True)
                kq8 = spool.tile([P, B], F8, tag="kq8")
                nc.vector.tensor_copy(kq8[:, :], kq_ps[:, :])

                sc_ps = pssc.tile([P, B, NT], F32, tag="sc")
                for b in range(B):
                    for t in range(NT):
                        nc.tensor.matmul(
                            sc_ps[:, b, t:t + 1],
                            xt_sb[j][:, b * W + t * P: b * W + (t + 1) * P],
                            kq8[:, b:b + 1],
                            start=True, stop=True,
                        )
                att_j = spool.tile([P, B, NT], F8, tag="att")
                for t in range(NT):
                    nc.scalar.activation(att_j[:, :, t], sc_ps[:, :, t],
                                         mybir.ActivationFunctionType.Exp,
                                         bias=bias_sb[:, t:t + 1],
                                         scale=1.0 / KSCALE)
                att.append(att_j)

            # Phase B per head: denominator, xa, output projection
            fin = spool.tile([P, HPC * B], F32, tag="fin", bufs=1)
            for j in range(HPC):
                b_ps = psb.tile([P, 3 * B], F32, tag="bps")
                den_ps = b_ps[:, 0:B]
                xa_ps = b_ps[:, B:2 * B]
                o_ps = b_ps[:, 2 * B:3 * B]
                nc.tensor.matmul(den_ps, ones8, att[j][:, :, 0],
                                 start=True, stop=False)
                nc.tensor.matmul(den_ps, ones8, att[j][:, :, 1],
                                 start=False, stop=True)
                rec = spool.tile([P, B], F32, tag="rec")
                nc.vector.reciprocal(rec[:, :], den_ps)

                for b in range(B):
                    for t in range(NT):
                        nc.tensor.matmul(
                            xa_ps[:, b:b + 1],
                            x_sb[j][:, b, t, :],             # [128, 128]
                            att[j][:, b, t:t + 1],           # [128, 1]
                            start=(t == 0), stop=(t == NT - 1),
                        )
                xa_bf = spool.tile([P, B], BF16, tag="xab")
                nc.vector.tensor_copy(xa_bf[:, :], xa_ps)

                nc.tensor.matmul(o_ps, wv[j], xa_bf[:, :], start=True, stop=True)
                t1 = spool.tile([P, B], F32, tag="t1")
                nc.vector.tensor_mul(t1[:, :], o_ps, rec[:, :])
                nc.gpsimd.tensor_add(fin[:, j * B:(j + 1) * B], t1[:, :],
                                     cnt_f[:, j * B:(j + 1) * B])
            # single output store once both heads' residual adds land
            nc.sync.dma_start(out=out_t[:, :], in_=fin[:, :])
    nc.finalize()
    return nc


def _get_nc():
    if "nc" not in _NC_CACHE:
        _NC_CACHE["nc"] = _build_nc()
    return _NC_CACHE["nc"]


def _pos_bias_f32():
    """t5_position_bucket exactly as the reference computes it, sliced to the
    window."""
    if "pos" not in _NC_CACHE:
        import jax.numpy as jnp
        NUM_BUCKETS, MAX_DISTANCE = 32, 128
        n = (S - 1) - jnp.arange(S)
        max_exact = NUM_BUCKETS // 2
        is_small = n < max_exact
        large = max_exact + (
            jnp.log(jnp.maximum(n, 1).astype(jnp.float32) / max_exact)
            / np.log(MAX_DISTANCE / max_exact)
            * (NUM_BUCKETS - max_exact)
        ).astype(jnp.int32)
        large = jnp.minimum(large, NUM_BUCKETS - 1)
        pos = jnp.where(is_small, n, large).astype(jnp.float32)
        _NC_CACHE["pos"] = np.asarray(pos)[CUTOFF:]  # [W]
    return _NC_CACHE["pos"]


def kernel(**inputs) -> np.ndarray:
    t = int(np.asarray(inputs["t"]))
    assert t == T, f"kernel hardcoded for t={T}, got {t}"
    content_t = np.ascontiguousarray(np.asarray(inputs["content_t"], dtype=np.float32))
    cache = np.asarray(inputs["cache"], dtype=np.float32)
    Wq = np.asarray(inputs["Wq"], dtype=np.float32)
    Wk = np.asarray(inputs["Wk"], dtype=np.float32)
    Wv = np.asarray(inputs["Wv"], dtype=np.float32)
    pos_param = np.float32(np.asarray(inputs["pos_param"]))

    pos = _pos_bias_f32()                                   # [W]
    posb = (-pos_param * pos).astype(np.float32)            # [W]
    bias_host = posb.reshape(NT, P).transpose(1, 0).astype(np.float32)  # [p, t]

    cnt_h = content_t.reshape(B, H, P)                      # [B, H, 128]
    # full window per (b, h): last 255 cache rows + content row
    win = np.empty((B, H, W, P), np.float32)
    win[:, :, : W - 1, :] = cache[:, CUTOFF:T, :].reshape(B, W - 1, H, P).transpose(0, 2, 1, 3)
    win[:, :, W - 1, :] = cnt_h
    win8 = win.astype(NP_F8)                                # [B, H, 256, 128] fp8

    # fold q and k projections + scaling into one matrix:
    #   kq = (KSCALE/sqrt(128)) * Wk_h (Wq_h^T cnt) = mfold_h^T cnt,
    #   mfold_h = (KSCALE/sqrt(128)) * Wq_h @ Wk_h^T
    mfold = np.einsum("hde,hfe->hdf", Wq, Wk) * np.float32(KSCALE / np.sqrt(128.0))

    in_maps = []
    for c in range(NCORES):
        h0 = HPC * c
        xt_host = np.empty((HPC, P, B * W), NP_F8)
        x_host = np.empty((HPC, P, B, NT, P), NP_F8)
        for j in range(HPC):
            wj = win8[:, h0 + j]                            # [B, 256, 128]
            xt_host[j] = wj.transpose(2, 0, 1).reshape(P, B * W)
            x_host[j] = wj.reshape(B, NT, P, P).transpose(2, 0, 1, 3)
        blocks = []
        for j in range(HPC):
            blocks += [mfold[h0 + j], Wv[h0 + j]]
        cntT = cnt_h[:, h0:h0 + HPC, :].transpose(2, 1, 0).reshape(P, HPC * B)
        blocks.append(cntT)
        cbf_host = np.concatenate(blocks, axis=1).astype(NP_BF16)
        cf_host = np.concatenate([bias_host, cntT], axis=1).astype(np.float32)
        in_maps.append({
            "xt": xt_host, "x": x_host, "cbf": cbf_host, "cf": cf_host,
        })

    nc = _get_nc()
    res = run_bass_kernel_spmd(nc, in_maps, list(range(NCORES)), **_RUN_KWARGS)
    _NC_CACHE["last_results"] = res
    outs = np.stack([np.asarray(res.results[c]["out"]) for c in range(NCORES)])
    # outs: [core, d, j*8+b] -> out_full[b, (2c+j)*128 + d]
    out_full = outs.reshape(NCORES, P, HPC, B).transpose(3, 0, 2, 1).reshape(B, H * P)
    return out_full.astype(np.float32)


_RUN_KWARGS = {}  # test harness may set {"trace": True, "tmpdir": ...}
